# revision 52
# baseline (speedup 1.0000x reference)
"""Causal self-attention (B=2, T=2048, C=1024, nh=16) on 8 TRN2 NeuronCores.

Sharding: core c -> batch b = c//4, head group g = c%4 (4 heads each).
Each core computes QKV projections for its heads, causal attention, and a
partial output projection (W_proj rows for its heads). The four partials per
batch are summed on the host, where b_proj is also added (no device bias).

Layouts (per core, hardcoded):
  xt   [128, 8, 2048]    x[b].T tiles:  xt[p, kt, t] = x[b, t, kt*128+p]
  wqk  [128, 8, 4, 128]  W_attn q|k cols for this core's heads
  wv   [128, 8, 256]     W_attn v cols
  bqk  [128, 4] f32      b_attn q|k (per-partition bias)
  bv   [128, 2] f32      b_attn v (per-partition bias)
  wp   [128, 2, 1024]    W_proj rows for this core's heads
  out  [2048, 1024] bf16 partial (x[b] @ ... for this head group)

In-kernel dataflow (all matmuls bf16 with fp32 PSUM accumulation), fully
interleaved per 512-token block tb so the PE streams continuously:
  qT,kT = (W.T @ x.T)      [feat, t] layout  (lhsT=W tile, rhs=xT)
  vT    = (W.T @ x.T), DMA-transposed per-tb into natural [t, feat] chunks
  S^T   = k @ q.T          [j, i] layout     (lhsT=kT tile, rhs=qT)
  P^T   = exp(S^T/8), masked on diagonal tiles (mult by 0/1 mask)
  y^T,l = [v|1].T @ P^T    [d, i] layout, row 64 = l = sum_j P
  yT    = y^T * (1/l broadcast)
  out   = yT.T @ Wp
Emission order: qkv0 qkv1 A0 qkv2 P0 A1 qkv3 P1 A2 P2 A3 P3 so the PE never
waits on the v-transpose or the softmax-normalize tail chains.
"""

import os
import sys

sys.path.insert(0, "/opt/trn_rl_repo")
os.environ.setdefault("MYCRO_LOCAL_CACHE", "1")

import ml_dtypes
import numpy as np

import concourse.bass as bass
import concourse.mybir as mybir
import concourse.tile as tile
from concourse import bacc
from concourse.bass_utils import run_bass_kernel_spmd

B, T, C, NH, HS = 2, 2048, 1024, 16, 64
HPC = 4  # heads per core
N_CORES = 8
KT = C // 128  # 8 contraction tiles over C
TT = T // 128  # 16 tiles over T
IB = T // 512  # 4 i-blocks over T
F32 = mybir.dt.float32

CD = mybir.dt.bfloat16
CD_NP = ml_dtypes.bfloat16

LAST_RESULT = None
_CACHE = {}


def _emit(nc, tc, ctx, aps):
    xt, wqk, wv, bqk, bvb, wp, out = (
        aps["xt"], aps["wqk"], aps["wv"], aps["bqk"], aps["bvb"], aps["wp"],
        aps["out"],
    )
    Exp = mybir.ActivationFunctionType.Exp

    consts = ctx.enter_context(tc.tile_pool(name="consts", bufs=1))

    # --- persistent SBUF tensors ---
    # xt tb0 as separate per-kt tiles so the first QKV matmuls depend only
    # on their own 128KB chunk landing; tb1-3 as one tile + one DMA config
    # each (sync-queue sequencer configs and semaphore-slot churn are the
    # scarce resource, and those blocks are prefetched anyway)
    xt_kt = [[consts.tile([128, 512], CD, tag=f"xt{kt}_{tb}",
                          name=f"xt{kt}_{tb}") for tb in range(IB)]
             for kt in range(KT)]

    def xt_rhs(tb, kt):
        return xt_kt[kt][tb][:]
    wqk_s = consts.tile([128, KT, 4, 128], CD, tag="wqk_s")
    wv_s = consts.tile([128, KT, 2, 128], CD, tag="wv_s")
    bqk_s = consts.tile([128, 4], F32, tag="bqk")
    bvb_s = consts.tile([128, 256], CD, tag="bvb")
    wp_s = consts.tile([128, 2, C], CD, tag="wp")

    # DMA order is the schedule: xt tb0 chunks first on the SP queue while
    # weights stream in parallel on the Activation HWDGE queue (idle at
    # startup), so the first QKV group can start as early as possible.
    # Weights arrive in half-tensor batches (few sequencer configs, arrival
    # order matches the v-first consumption order). Biases are only needed
    # at the first PSUM drain, so they go after the tb0 wave.
    nc.scalar.dma_start(wv_s[:, 0:4], wv[:, 0:4])
    nc.scalar.dma_start(wv_s[:, 4:8], wv[:, 4:8])
    nc.scalar.dma_start(wqk_s[:, 0:4], wqk[:, 0:4])
    nc.scalar.dma_start(wqk_s[:, 4:8], wqk[:, 4:8])
    for kt in range(KT):
        nc.sync.dma_start(xt_kt[kt][0][:], xt[:, kt, 0:512])
    nc.sync.dma_start(bqk_s[:], bqk)
    nc.sync.dma_start(bvb_s[:], bvb)
    for kt in range(KT):
        nc.scalar.dma_start(xt_kt[kt][1][:], xt[:, kt, 512:1024])
    nc.scalar.dma_start(wp_s[:], wp)

    def emit_xt_dma(tb):
        for kt in range(KT):
            nc.sync.dma_start(
                xt_kt[kt][tb][:],
                xt[:, kt, tb * 512:(tb + 1) * 512],
            )

    # kT per head, zero-padded to full 128 partitions: head h occupies rows
    # (h%2)*64..+64, only the complementary 64 rows need zeroing. Full-K
    # S-matmuls keep the PE HAM clock-gate warm.
    kz_t = [consts.tile([128, T], CD, tag=f"kz{h}", name=f"kz{h}")
            for h in range(HPC)]
    for h in range(HPC):
        if h % 2 == 0:
            nc.gpsimd.memset(kz_t[h][64:128, :], 0.0)
        else:
            nc.gpsimd.memset(kz_t[h][0:64, :], 0.0)

    vext_s = consts.tile([128, TT, HPC * (HS + 1)], CD, tag="vext")
    vext4 = vext_s[:].rearrange("p t (h c) -> p t h c", c=HS + 1)
    nc.gpsimd.memset(vext4[:, :, :, HS], 1.0)  # ones columns

    # causal mask, shifted-triangle trick: mask[j, c] = 1 if j <= c - 384.
    mask_s = consts.tile([128, 896], CD, tag="mask")
    nc.vector.memset(mask_s[:], 1.0)
    nc.gpsimd.affine_select(
        out=mask_s[:],
        in_=mask_s[:],
        compare_op=mybir.AluOpType.is_ge,
        fill=0.0,
        base=-384,
        channel_multiplier=-1,
        pattern=[[1, 896]],
    )
    # fixed 128x128 causal triangle (j' <= c') for diagonal strips
    tri = mask_s[:, 384:512]

    qk_t = [consts.tile([128, T], CD, tag=f"q{jt}", name=f"q{jt}")
            for jt in range(2)]
    yt_s = consts.tile([128, 2, T], CD, tag="yt")

    # --- PSUM pools: pp(1 bank)x2 + S(2 banks)x2 + y(1 bank)x2 = 8 banks ---
    pp = ctx.enter_context(tc.tile_pool(name="pp", bufs=2, space="PSUM"))
    attn_sp = ctx.enter_context(tc.tile_pool(name="attn_s", bufs=2, space="PSUM"))
    attn_yp = ctx.enter_context(tc.tile_pool(name="attn_y", bufs=2, space="PSUM"))
    pt_pool = ctx.enter_context(tc.tile_pool(name="pt", bufs=8))
    misc = ctx.enter_context(tc.tile_pool(name="misc", bufs=8))
    stage = ctx.enter_context(tc.tile_pool(name="stage", bufs=3))
    # fp32 half-proj partials for the split final-block projection
    stage_f = ctx.enter_context(tc.tile_pool(name="stage_f", bufs=8))

    def qkv_drain(tb, jt, ps):
        tsl = slice(tb * 512, (tb + 1) * 512)
        if jt < 2:  # q
            nc.vector.tensor_scalar_add(
                qk_t[jt][:, tsl], ps[:], bqk_s[:, jt:jt + 1]
            )
        else:  # k -> zero-padded per-head kz
            nc.vector.tensor_scalar_add(
                kz_t[2 * (jt - 2)][0:64, tsl],
                ps[0:64, :],
                bqk_s[0:64, jt:jt + 1],
            )
            nc.vector.tensor_scalar_add(
                kz_t[2 * (jt - 2) + 1][64:128, tsl],
                ps[64:128, :],
                bqk_s[64:128, jt:jt + 1],
            )

    def qkv_units(tb):
        # one closure per matmul so QKV groups can be dribbled between
        # attention pairs; each group-closing unit also emits its drain.
        # v runs x-stationary straight into natural [t, feat] layout (no
        # transpose chain), one 128-token column block at a time, and its
        # drain adds the bias row and interleaves into vext. v first so
        # vext is ready well before the next attention block needs it.
        units = []
        for tloc in range(4):
            state = {}
            ttp = tb * 4 + tloc
            for kt in range(KT):
                def unit(tb=tb, tloc=tloc, ttp=ttp, kt=kt, state=state):
                    if kt == 0:
                        state["ps"] = pp.tile([128, 512], F32, tag="pp", name="ps")
                    ps = state["ps"]
                    nc.tensor.matmul(
                        out=ps[:, 0:256],
                        lhsT=xt_rhs(tb, kt)[:, tloc * 128:(tloc + 1) * 128],
                        rhs=wv_s[:, kt],
                        start=(kt == 0),
                        stop=(kt == KT - 1),
                    )
                    if kt == KT - 1:
                        nc.vector.tensor_add(
                            vext4[:, ttp, :, 0:HS],
                            ps[:, 0:256].rearrange("p (h c) -> p h c", c=HS),
                            bvb_s[:].rearrange("p (h c) -> p h c", c=HS),
                        )
                units.append(unit)
        for jt in (2, 3, 0, 1):  # k before q
            state = {}
            for kt in range(KT):
                def unit(tb=tb, jt=jt, kt=kt, state=state):
                    if kt == 0:
                        state["ps"] = pp.tile([128, 512], F32, tag="pp", name="ps")
                    ps = state["ps"]
                    nc.tensor.matmul(
                        out=ps[:],
                        lhsT=wqk_s[:, kt, jt, :],
                        rhs=xt_rhs(tb, kt),
                        start=(kt == 0),
                        stop=(kt == KT - 1),
                    )
                    if kt == KT - 1:
                        qkv_drain(tb, jt, ps)
                units.append(unit)
        return units

    def proj_units(ib, final=False):
        units = []
        for tloc in range(4):
            ttp = ib * 4 + tloc
            for eb in range(2):
                def unit(ttp=ttp, eb=eb):
                    psp = pp.tile([128, 512], F32, tag="pp", name="psp")
                    for dt in range(2):
                        nc.tensor.matmul(
                            out=psp[:],
                            lhsT=yt_s[:, dt, ttp * 128:(ttp + 1) * 128],
                            rhs=wp_s[:, dt, eb * 512:(eb + 1) * 512],
                            start=(dt == 0),
                            stop=(dt == 1),
                        )
                    st = stage.tile([128, 512], CD, tag="st", name="st")
                    # Scalar does PSUM->SBUF casts only in the final proj
                    # block, where it has no exp work left to delay
                    if final and eb == 1:
                        nc.scalar.copy(st[:], psp[:])
                    else:
                        nc.vector.tensor_copy(st[:], psp[:])
                    nc.sync.dma_start(
                        out[ttp * 128:(ttp + 1) * 128,
                            eb * 512:(eb + 1) * 512],
                        st[:],
                    )
                units.append(unit)
        return units

    def proj_split_units(ib):
        """Final-block projection split by contraction half: the dt=0
        matmuls only need heads 0/1 and run as fillers inside the last
        attention block's tail heads; dt=1 + combine is all that's left
        after the last head, shrinking the kernel's serial tail."""
        units_a, units_b = [], []
        stf_t = {}
        for tloc in range(4):
            ttp = ib * 4 + tloc
            for eb in range(2):
                def unit_a(ttp=ttp, eb=eb):
                    psp = pp.tile([128, 512], F32, tag="pp", name="psp")
                    nc.tensor.matmul(
                        out=psp[:],
                        lhsT=yt_s[:, 0, ttp * 128:(ttp + 1) * 128],
                        rhs=wp_s[:, 0, eb * 512:(eb + 1) * 512],
                        start=True,
                        stop=True,
                    )
                    stf = stage_f.tile([128, 512], F32, tag="stf", name="stf")
                    stf_t[(ttp, eb)] = stf
                    nc.vector.tensor_copy(stf[:], psp[:])

                def unit_b(ttp=ttp, eb=eb):
                    psp = pp.tile([128, 512], F32, tag="pp", name="psp")
                    nc.tensor.matmul(
                        out=psp[:],
                        lhsT=yt_s[:, 1, ttp * 128:(ttp + 1) * 128],
                        rhs=wp_s[:, 1, eb * 512:(eb + 1) * 512],
                        start=True,
                        stop=True,
                    )
                    st = stage.tile([128, 512], CD, tag="st", name="st")
                    nc.vector.tensor_add(st[:], psp[:], stf_t[(ttp, eb)][:])
                    nc.sync.dma_start(
                        out[ttp * 128:(ttp + 1) * 128,
                            eb * 512:(eb + 1) * 512],
                        st[:],
                    )
                units_a.append(unit_a)
                units_b.append(unit_b)
        return units_a, units_b

    filler = []

    def pop_fillers(n):
        for _ in range(min(n, len(filler))):
            filler.pop(0)()

    def emit_attn(ib, head_extra=None):
        isl = slice(ib * 512, (ib + 1) * 512)
        npairs = (2 * ib + 2) * HPC
        done = 0
        for h in range(HPC):
            if head_extra and h in head_extra:
                filler.extend(head_extra[h])
            jt_q = h // 2
            row = (h % 2) * 64
            psy = attn_yp.tile([HS + 1, 512], F32, tag="y")
            njt = 4 * ib + 4
            for j0 in range(0, njt, 2):
                grp = (j0, j0 + 1)
                offs = [max(0, 128 * j - 512 * ib) for j in grp]
                ws = [512 - o for o in offs]
                cs = [0, ws[0]]  # narrowed tiles pack contiguously in PSUM
                wflat = ws[0] + ws[1]
                psS = attn_sp.tile([128, 1024], F32, tag="s")
                pt = pt_pool.tile([128, 1024], CD, tag="pt")
                for gi, j in enumerate(grp):
                    nc.tensor.matmul(
                        out=psS[:, cs[gi]: cs[gi] + ws[gi]],
                        lhsT=kz_t[h][:, j * 128:(j + 1) * 128],
                        rhs=qk_t[jt_q][:, ib * 512 + offs[gi]:(ib + 1) * 512],
                        start=True,
                        stop=True,
                    )
                nc.scalar.activation(
                    out=pt[:, 0:wflat], in_=psS[:, 0:wflat],
                    func=Exp, scale=0.125,
                )
                # dribble QKV/proj matmuls here: they execute on the PE
                # while the Scalar engine computes this pair's exp
                rem = npairs - done
                pop_fillers(-(-len(filler) // rem) if rem else len(filler))
                done += 1
                for gi, j in enumerate(grp):
                    if 128 * j >= 512 * ib:  # diagonal tile -> mask boundary
                        nc.vector.tensor_mul(
                            pt[:, cs[gi]:cs[gi] + 128],
                            pt[:, cs[gi]:cs[gi] + 128],
                            tri,
                        )
                    nc.tensor.matmul(
                        out=psy[:, offs[gi]:512],
                        lhsT=vext4[:, j, h, :],
                        rhs=pt[:, cs[gi]:cs[gi] + ws[gi]],
                        start=(j == 0),
                        stop=(j == njt - 1),
                    )
            # softmax denominator: copy out the [1,512] l-row, invert it
            # (cheap), then broadcast the reciprocal across 64 partitions.
            # Broadcast+multiply run in 256-col halves so the first half of
            # yt (and anything waiting on it) is ready ~1us sooner.
            lrow = misc.tile([1, 512], F32, tag="lrow")
            nc.vector.tensor_copy(lrow[:], psy[HS:HS + 1, :])
            linv1 = misc.tile([1, 512], F32, tag="linv1")
            nc.vector.reciprocal_approx_fast(linv1[:], lrow[:])
            lbc = misc.tile([64, 512], F32, tag="lbc")
            for half in range(2):
                hs_ = slice(half * 256, (half + 1) * 256)
                nc.gpsimd.partition_broadcast(
                    lbc[:, hs_], linv1[:, hs_], channels=64
                )
                nc.vector.tensor_mul(
                    yt_s[row:row + 64, jt_q,
                         ib * 512 + half * 256:ib * 512 + (half + 1) * 256],
                    psy[0:HS, hs_], lbc[:, hs_]
                )

    # tb0 QKV runs upfront (nothing to hide it behind); everything after
    # is dribbled between attention pairs
    for u in qkv_units(0):
        u()
    for ib in range(IB):
        if ib + 2 < IB:
            emit_xt_dma(ib + 2)
        if ib < IB - 1:
            q_units = qkv_units(ib + 1)
        else:
            q_units = []
        # all deferred proj fillers go to A3, the only block with no QKV
        # work left to hide exp behind
        if ib == 3:
            p_units = proj_units(0) + proj_units(1) + proj_units(2)
        else:
            p_units = []
        # interleave ~3:1 qkv:proj (qkv is the more urgent dependency)
        mixed = []
        qi = pi = 0
        while qi < len(q_units) or pi < len(p_units):
            for _ in range(3):
                if qi < len(q_units):
                    mixed.append(q_units[qi])
                    qi += 1
            if pi < len(p_units):
                mixed.append(p_units[pi])
                pi += 1
        filler.extend(mixed)
        if ib == IB - 1:
            p3a, p3b = proj_split_units(IB - 1)
            emit_attn(ib, head_extra={2: p3a})
            pop_fillers(len(filler))
            for u in p3b:
                u()
        else:
            emit_attn(ib)
            pop_fillers(len(filler))


def _enable_ldw_opt():
    """Flip walrus's --enable-ldw-opt to true: hoists/merges LDWEIGHTS so
    back-to-back matmuls don't each pay the ~53ns weight-load bubble."""
    import concourse.bass_utils as _bu

    if getattr(_bu, "_ldw_patched", False):
        return
    orig = _bu.run_command

    def patched(cmd, *a, **kw):
        cmd = ["--enable-ldw-opt=true" if c == "--enable-ldw-opt=false" else c
               for c in cmd]
        return orig(cmd, *a, **kw)

    _bu.run_command = patched
    _bu._ldw_patched = True


def build():
    if "nc" in _CACHE:
        return _CACHE["nc"]
    if os.environ.get("KERNEL_LDW_OPT", "0") == "1":
        # walrus codegen rejects this on TRN2 (visitInstLdweights error);
        # kept behind an env flag for experiments only
        _enable_ldw_opt()
    nc = bacc.Bacc(
        "TRN2", target_bir_lowering=False, debug=False, num_devices=N_CORES
    )
    aps = {
        "xt": nc.dram_tensor("xt", [128, KT, T], CD, kind="ExternalInput").ap(),
        "wqk": nc.dram_tensor("wqk", [128, KT, 4, 128], CD, kind="ExternalInput").ap(),
        "wv": nc.dram_tensor("wv", [128, KT, 2, 128], CD, kind="ExternalInput").ap(),
        "bqk": nc.dram_tensor("bqk", [128, 4], F32, kind="ExternalInput").ap(),
        "bvb": nc.dram_tensor("bvb", [128, 256], CD, kind="ExternalInput").ap(),
        "wp": nc.dram_tensor("wp", [128, 2, C], CD, kind="ExternalInput").ap(),
        "out": nc.dram_tensor("out", [T, C], CD, kind="ExternalOutput").ap(),
    }
    from contextlib import ExitStack

    with tile.TileContext(nc) as tc:
        with ExitStack() as ctx:
            _emit(nc, tc, ctx, aps)
    nc.compile()
    _CACHE["nc"] = nc
    return nc


def make_in_maps(x, W_attn, b_attn, W_proj, b_proj):
    x = np.asarray(x, dtype=np.float32)
    W_attn = np.asarray(W_attn, dtype=np.float32)
    b_attn = np.asarray(b_attn, dtype=np.float32)
    W_proj = np.asarray(W_proj, dtype=np.float32)

    in_maps = []
    xt_b = {}
    for b in range(B):
        xt = np.ascontiguousarray(x[b].T)  # [C, T]
        xt_b[b] = (
            xt.reshape(KT, 128, T).transpose(1, 0, 2).astype(CD_NP)
        )
    for core in range(N_CORES):
        b = core // 4
        g = core % 4
        fs = slice(256 * g, 256 * g + 256)  # feature cols for this head group
        wq = W_attn[:, fs]
        wk = W_attn[:, C + 256 * g: C + 256 * g + 256]
        wv = W_attn[:, 2 * C + 256 * g: 2 * C + 256 * g + 256]
        wqk = np.concatenate([wq, wk], axis=1)  # [1024, 512]
        bq = b_attn[fs]
        bk = b_attn[C + 256 * g: C + 256 * g + 256]
        bv = b_attn[2 * C + 256 * g: 2 * C + 256 * g + 256]
        in_maps.append({
            "xt": xt_b[b],
            "wqk": np.ascontiguousarray(
                wqk.reshape(KT, 128, 4, 128).transpose(1, 0, 2, 3)
            ).astype(CD_NP),
            "wv": np.ascontiguousarray(
                wv.reshape(KT, 128, 2, 128).transpose(1, 0, 2, 3)
            ).astype(CD_NP),
            "bqk": np.ascontiguousarray(
                np.concatenate([bq, bk]).reshape(4, 128).T
            ).astype(np.float32),
            "bvb": np.ascontiguousarray(
                np.broadcast_to(bv[None, :], (128, 256))
            ).astype(CD_NP),
            "wp": np.ascontiguousarray(
                W_proj[fs, :].reshape(2, 128, C).transpose(1, 0, 2)
            ).astype(CD_NP),
        })
    return in_maps


def _ensure_ntff_hook():
    """Recreate the missing antenv.axon_hooks NTFF-profile shim (see
    trn_agent_boot/trn_boot.py) so run_bass_kernel_spmd(trace=True) works."""
    import contextlib
    import ctypes
    import types

    try:
        from antenv.axon_hooks import get_axon_ntff_profile_hook  # noqa: F401

        return
    except ImportError:
        pass

    mod = types.ModuleType("antenv.axon_hooks")
    _holder = {"hook": None}
    mod.set_axon_ntff_profile_hook = lambda h: _holder.__setitem__("hook", h)
    mod.get_axon_ntff_profile_hook = lambda: _holder["hook"]
    sys.modules["antenv.axon_hooks"] = mod
    import antenv

    antenv.axon_hooks = mod

    so_path = "/opt/axon/libaxon_pjrt.so"
    if not os.path.exists(so_path):
        return
    lib = ctypes.CDLL(so_path)
    if not hasattr(lib, "axon_start_nrt_profile"):
        return
    lib.axon_start_nrt_profile.argtypes = [
        ctypes.POINTER(ctypes.c_int64),
        ctypes.c_size_t,
    ]
    lib.axon_start_nrt_profile.restype = ctypes.c_int64
    lib.axon_stop_nrt_profile.argtypes = [ctypes.c_char_p]
    lib.axon_stop_nrt_profile.restype = ctypes.c_int64

    @contextlib.contextmanager
    def _hook(output_dir, device_ids):
        import jax

        jax.devices()
        if device_ids:
            ids = (ctypes.c_int64 * len(device_ids))(*device_ids)
            rc = lib.axon_start_nrt_profile(ids, len(device_ids))
        else:
            rc = lib.axon_start_nrt_profile(None, 0)
        if rc != 0:
            raise RuntimeError(f"axon_start_nrt_profile rc={rc}")
        try:
            yield
        finally:
            n = lib.axon_stop_nrt_profile(str(output_dir).encode())
            if n <= 0:
                print(f"ntff profile: rc={n}, nothing written to {output_dir}")

    mod.set_axon_ntff_profile_hook(_hook)


def kernel(x, W_attn, b_attn, W_proj, b_proj):
    global LAST_RESULT
    b_proj = np.asarray(b_proj, dtype=np.float32)
    nc = build()
    in_maps = make_in_maps(x, W_attn, b_attn, W_proj, b_proj)
    trace = os.environ.get("KERNEL_TRACE", "0") == "1"
    if trace:
        _ensure_ntff_hook()
        import concourse.bass_utils as _bu

        _bu.upload_artifacts = lambda tmpdir: f"local://{tmpdir}"
    res = run_bass_kernel_spmd(
        nc, in_maps, core_ids=list(range(N_CORES)), trace=trace
    )
    LAST_RESULT = res
    outs = [res.results[i]["out"].astype(np.float32) for i in range(N_CORES)]
    y = np.empty((B, T, C), dtype=np.float32)
    for b in range(B):
        y[b] = (
            outs[4 * b] + outs[4 * b + 1] + outs[4 * b + 2] + outs[4 * b + 3]
            + b_proj
        )
    return y
